# revision 1
# baseline (speedup 1.0000x reference)
"""Complex self-attention on 8 Trainium2 NeuronCores (Bass/Tile).

Reference computation (B=2, S=2048, F=1024, H=16, D=64):
    Q/K/V = complex_linear(x, W{q,k,v});  scores = Re(Q K^H) * D^-0.5
    attn = softmax(scores + mask_bias);  out = complex_linear(attn @ V, Wo)
    return stack([out_r, out_i])            # [2, B, S, F]

Sharding: 8 cores = 2 batches x 4 head-groups (4 heads each). Each core
computes its heads' Q/K/V projections, the attention, and a *partial*
output projection (contraction over its 256 features of Wo); the host
sums the 4 partials per batch and adds bo (the collective).

Complex arithmetic is folded into real matmuls by stacking (re, im)
parts along the contraction axis with host-prepped weight layouts:
    Xcat^T = [x_r^T ; x_i^T]   [2F, S]
    WQ[:, c<64]  = [Wq_r ; -Wq_i] col,  WQ[:, c>=64] = [Wq_i ; Wq_r] col
so one f32r matmul chain yields (Qr | Qi) per head, and head-local
tiles [128, *] carry (re 0:64, im 64:128) on the partition axis.

Softmax: no max subtraction (scaled scores have sigma~1.4, |s|<9, exp
is safe in f32); denominator comes from an appended mask column on the
V tiles, so the attn @ V matmul also yields sum_k exp * mask; division
happens per-partition in the natural [q, d] layout.
"""

import sys

if "/opt/trn_rl_repo" not in sys.path:
    sys.path.insert(0, "/opt/trn_rl_repo")

import numpy as np
import ml_dtypes

B, S, F = 2, 2048, 1024
H, D = 16, 64
NCORES = 8
HL = 4           # heads per core
D2 = 2 * D       # 128 = (re|im) feature rows per head
G = 2 * F // 128  # 16 contraction tiles over [x_r; x_i]
NST = S // 128    # 16 query/key 128-tiles
NQB = S // 512    # 4 query 512-blocks

BF16 = ml_dtypes.bfloat16

_CACHE = {}


def _build_program(with_bias=False):
    from concourse import bass, bacc, mybir, tile

    F32 = mybir.dt.float32
    F32R = mybir.dt.float32r
    BF = mybir.dt.bfloat16
    EXP = mybir.ActivationFunctionType.Exp

    nc = bacc.Bacc("TRN2", target_bir_lowering=False, debug=False)

    xcat = nc.dram_tensor("xcat", [2 * F, S], F32R, kind="ExternalInput")
    # Karatsuba weight blocks: [F, 3, HL*D] with m in {Wr, Wi, Wr+Wi}
    WK3 = 3 * HL * D
    wq_d = nc.dram_tensor("wq", [F, WK3], F32R, kind="ExternalInput")
    wk_d = nc.dram_tensor("wk", [F, WK3], F32R, kind="ExternalInput")
    wv_d = nc.dram_tensor("wv", [F, WK3], F32R, kind="ExternalInput")
    wor_d = nc.dram_tensor("wor", [HL * D2, F], BF, kind="ExternalInput")
    woi_d = nc.dram_tensor("woi", [HL * D2, F], BF, kind="ExternalInput")
    bq_d = nc.dram_tensor("bqrep", [128, HL * D2], BF, kind="ExternalInput")
    bk_d = nc.dram_tensor("bkrep", [128, HL * D2], BF, kind="ExternalInput")
    bv_d = nc.dram_tensor("bvrep", [128, HL * D2], BF, kind="ExternalInput")
    mask_d = nc.dram_tensor("maskcols", [128, NST], F32, kind="ExternalInput")
    ident_d = nc.dram_tensor("ident", [128, 128], F32R, kind="ExternalInput")
    out_r = nc.dram_tensor("out_r", [S, F], F32, kind="ExternalOutput")
    out_i = nc.dram_tensor("out_i", [S, F], F32, kind="ExternalOutput")

    scale = 1.0 / float(np.sqrt(D))

    with tile.TileContext(nc) as tc, nc.allow_low_precision("f32r/bf16 pipeline"):
        with (
            tc.tile_pool(name="consts", bufs=1) as cpool,
            tc.tile_pool(name="qkt", bufs=1) as qkt_pool,
            tc.tile_pool(name="vaug", bufs=1) as vaug_pool,
        ):
            ident = cpool.tile([128, 128], F32R)
            nc.sync.dma_start(ident[:], ident_d.ap())
            ident_bf = cpool.tile([128, 128], BF)
            nc.gpsimd.dma_start(ident_bf[:], ident_d.ap().bitcast(F32))
            mask_sb = cpool.tile([128, NST], F32)
            nc.sync.dma_start(mask_sb[:], mask_d.ap())
            bq_sb = cpool.tile([128, HL * D2], BF)
            nc.sync.dma_start(bq_sb[:], bq_d.ap())
            bk_sb = cpool.tile([128, HL * D2], BF)
            nc.sync.dma_start(bk_sb[:], bk_d.ap())
            bv_sb = cpool.tile([128, HL * D2], BF)
            nc.sync.dma_start(bv_sb[:], bv_d.ap())

            # Resident activations: transposed Q/K (f32r) and masked V (+mask col, bf16)
            qt = qkt_pool.tile([128, HL, S], F32R)   # [d_ri, h, s]
            kt = qkt_pool.tile([128, HL, S], F32R)
            va = vaug_pool.tile([128, HL, NST, D2 + 1], BF)  # [k, h, k_tile, d_ri|mask]

            # ---------------- Phase 1a: Q,K projections (Karatsuba), transpose
            # m1 = xr@Wr, m2 = xi@Wi, m3 = (xr+xi)@(Wr+Wi);
            # re = m1-m2, im = m3-m1-m2. 3 matmul chains instead of 4.
            # wv lives in its own pool so its prefetch DMAs (paced through the
            # 1a loop) land in SBUF that does not overlap the 1a pools.
            GF = F // 128  # 8 contraction tiles per m-chain
            HD = HL * D    # 256 columns per m-chain
            wvpool = tc.alloc_tile_pool(name="wv", bufs=1)
            wv_sb = wvpool.tile([128, GF, WK3], F32R)
            with (
                tc.tile_pool(name="wqk", bufs=1) as wpool,
                tc.tile_pool(name="xcol", bufs=3) as xpool,
                tc.tile_pool(name="xsum", bufs=2) as xsum_pool,
                tc.tile_pool(name="stage", bufs=2) as spool,
                tc.tile_pool(name="proj_ps", bufs=6, space="PSUM") as proj_ps,
                tc.tile_pool(name="tr_ps", bufs=2, space="PSUM") as tr_ps,
            ):
                wq_sb = wpool.tile([128, GF, WK3], F32R, tag="wq")
                wk_sb = wpool.tile([128, GF, WK3], F32R, tag="wk")
                WCH = 2  # g-tiles per weight-DMA chunk

                def load_w_chunk(w_sb, w_d, c):
                    nc.sync.dma_start(
                        w_sb[:, c : c + WCH, :],
                        w_d.ap()[c * 128 : (c + WCH) * 128, :].rearrange(
                            "(g p) n -> p g n", p=128
                        ),
                    )

                def proj_chains(ps_pool, xlo, xhi, xs, w_sb, tag):
                    # contiguous chains: m1 needs only xlo + early w chunks, so
                    # the PE starts before xs (DVE add) and late chunks land
                    m1 = ps_pool.tile([128, HD], F32, tag="pm", name=f"{tag}_m1")
                    m2 = ps_pool.tile([128, HD], F32, tag="pm", name=f"{tag}_m2")
                    m3 = ps_pool.tile([128, HD], F32, tag="pm", name=f"{tag}_m3")
                    for m, x_sb, c0 in ((m1, xlo, 0), (m2, xhi, HD), (m3, xs, 2 * HD)):
                        for g in range(GF):
                            nc.tensor.matmul(m[:], x_sb[:, g, :],
                                             w_sb[:, g, c0 : c0 + HD],
                                             start=(g == 0), stop=(g == GF - 1))
                    return m1, m2, m3

                def hd_view(ap2d):
                    return ap2d.rearrange("p (h d) -> p h d", d=D)

                def combine_nat(m1, m2, m3, nat, tmp, c2, b_sb):
                    # nat[:, h*128+(0:64)] = m1-m2 ; nat[:, h*128+(64:128)] = m3-m1-m2
                    # (walrus: TT reads at most one PSUM operand -> stage m2)
                    natv = nat[:].rearrange("p (h c) -> p h c", c=D2)
                    nc.vector.tensor_copy(c2[:], m2[:])
                    nc.vector.tensor_sub(natv[:, :, 0:D], hd_view(m1[:]), hd_view(c2[:]))
                    nc.vector.tensor_sub(tmp[:], m3[:], c2[:])
                    nc.vector.tensor_sub(natv[:, :, D:D2], hd_view(tmp[:]), hd_view(m1[:]))
                    if with_bias:
                        nc.vector.tensor_add(nat[:], nat[:], b_sb[:])

                load_w_chunk(wq_sb, wq_d, 0)
                load_w_chunk(wq_sb, wq_d, 2)

                for st in range(NST):
                    xlo = xpool.tile([128, GF, 128], F32R, tag="xlo", name="xlo")
                    nc.sync.dma_start(
                        xlo[:],
                        xcat.ap()[0:F, st * 128 : (st + 1) * 128].rearrange(
                            "(g p) m -> p g m", p=128
                        ),
                    )
                    xhi = xpool.tile([128, GF, 128], F32R, tag="xhi", name="xhi")
                    nc.sync.dma_start(
                        xhi[:],
                        xcat.ap()[F : 2 * F, st * 128 : (st + 1) * 128].rearrange(
                            "(g p) m -> p g m", p=128
                        ),
                    )
                    if st == 0:
                        # grouped by tensor: PE consumes Q chains first, then K, V
                        for w_sb, w_d in ((wq_sb, wq_d), (wk_sb, wk_d), (wv_sb, wv_d)):
                            for c in range(4, GF, WCH) if w_sb is wq_sb else range(0, GF, WCH):
                                load_w_chunk(w_sb, w_d, c)
                    xs = xsum_pool.tile([128, GF, 128], F32R, name="xs")
                    nc.vector.tensor_add(xs[:], xlo[:], xhi[:])

                    q_m = proj_chains(proj_ps, xlo, xhi, xs, wq_sb, "q")
                    qn = spool.tile([128, HL * D2], F32R, tag="nat", name="qn")
                    tmp = spool.tile([128, HD], F32, tag="tmp", name="tmpq")
                    c2q = spool.tile([128, HD], F32, tag="c2", name="c2q")
                    combine_nat(*q_m, qn, tmp, c2q, bq_sb)
                    k_m = proj_chains(proj_ps, xlo, xhi, xs, wk_sb, "k")
                    kn = spool.tile([128, HL * D2], F32R, tag="nat", name="kn")
                    tmp2 = spool.tile([128, HD], F32, tag="tmp", name="tmpk")
                    c2k = spool.tile([128, HD], F32, tag="c2", name="c2k")
                    combine_nat(*k_m, kn, tmp2, c2k, bk_sb)
                    for h in range(HL):
                        qtp = tr_ps.tile([128, 128], F32R, tag="tr")
                        nc.tensor.transpose(
                            qtp[:], qn[:, h * D2 : (h + 1) * D2], ident[:]
                        )
                        nc.vector.tensor_copy(
                            qt[:, h, st * 128 : (st + 1) * 128], qtp[:]
                        )
                        ktp = tr_ps.tile([128, 128], F32R, tag="tr")
                        nc.tensor.transpose(
                            ktp[:], kn[:, h * D2 : (h + 1) * D2], ident[:]
                        )
                        nc.vector.tensor_copy(
                            kt[:, h, st * 128 : (st + 1) * 128], ktp[:]
                        )
                    # V chains share the same psum slots (freed by Q combines)
                    v_m1, v_m2, v_m3 = proj_chains(proj_ps, xlo, xhi, xs, wv_sb, "v")
                    c2v = spool.tile([128, HD], F32, tag="c2", name="c2v")
                    nc.vector.tensor_copy(c2v[:], v_m2[:])
                    t_re = spool.tile([128, HD], F32, tag="vt", name="t_re")
                    nc.vector.tensor_sub(t_re[:], v_m1[:], c2v[:])
                    t_im = spool.tile([128, HD], F32, tag="vt", name="t_im")
                    nc.vector.tensor_sub(t_im[:], v_m3[:], c2v[:])
                    nc.vector.tensor_sub(t_im[:], t_im[:], v_m1[:])
                    if with_bias:
                        bv_v = bv_sb[:].rearrange("p (h c) -> p h c", c=D2)
                        t_re_v = t_re[:].rearrange("p (h d) -> p h d", d=D)
                        t_im_v = t_im[:].rearrange("p (h d) -> p h d", d=D)
                        nc.vector.tensor_add(t_re_v[:], t_re_v[:], bv_v[:, :, 0:D])
                        nc.vector.tensor_add(t_im_v[:], t_im_v[:], bv_v[:, :, D:D2])
                    COPY = mybir.ActivationFunctionType.Copy
                    for h in range(HL):
                        nc.vector.tensor_scalar_mul(
                            va[:, h, st, 0:D],
                            t_re[:, h * D : (h + 1) * D],
                            mask_sb[:, st : st + 1],
                        )
                        nc.vector.tensor_scalar_mul(
                            va[:, h, st, D:D2],
                            t_im[:, h * D : (h + 1) * D],
                            mask_sb[:, st : st + 1],
                        )
                        nc.vector.tensor_copy(
                            va[:, h, st, D2 : D2 + 1], mask_sb[:, st : st + 1]
                        )

            wvpool.release()

            # -------- Phase 2+3: attention (1024-wide q blocks) + O-proj ------
            # Loop order: q-block outer, head inner; after all heads of a
            # q-block finish, that block's output projection runs — its PE
            # matmuls fill the ACT-bound (exp) stretches of the next block.
            QW = 512  # q-block width
            with (
                tc.tile_pool(name="atp", bufs=1) as at_pool,
                tc.tile_pool(name="wo", bufs=1) as wopool,
                tc.tile_pool(name="p_sb", bufs=2 * NST + 2) as p_pool,
                tc.tile_pool(name="asb", bufs=4) as a_pool,
                tc.tile_pool(name="rcp", bufs=4) as r_pool,
                tc.tile_pool(name="ost", bufs=4) as opool,
                tc.tile_pool(name="sc_ps", bufs=4, space="PSUM") as sc_ps,
                tc.tile_pool(name="a_ps", bufs=2, space="PSUM") as a_ps,
                tc.tile_pool(name="o_ps", bufs=2, space="PSUM") as o_ps,
            ):
                at = at_pool.tile([128, HL, S], BF)  # [d_ri, h, s] attn out^T
                wor_sb = wopool.tile([128, HL, F], BF, tag="wor")
                nc.sync.dma_start(
                    wor_sb[:], wor_d.ap().rearrange("(h p) n -> p h n", p=128)
                )
                woi_sb = wopool.tile([128, HL, F], BF, tag="woi")
                nc.sync.dma_start(
                    woi_sb[:], woi_d.ap().rearrange("(h p) n -> p h n", p=128)
                )
                def oproj_block(st):
                    # output projection for s rows [st*128, (st+1)*128)
                    for fo in range(2):
                        opr = o_ps.tile([128, 512], F32, tag="o")
                        opi = o_ps.tile([128, 512], F32, tag="o")
                        for h2 in range(HL):
                            lhsT = at[:, h2, st * 128 : (st + 1) * 128]
                            nc.tensor.matmul(
                                opr[:], lhsT, wor_sb[:, h2, fo * 512 : (fo + 1) * 512],
                                start=(h2 == 0), stop=(h2 == HL - 1),
                            )
                            nc.tensor.matmul(
                                opi[:], lhsT, woi_sb[:, h2, fo * 512 : (fo + 1) * 512],
                                start=(h2 == 0), stop=(h2 == HL - 1),
                            )
                        for ops, dram in ((opr, out_r), (opi, out_i)):
                            osb = opool.tile([128, 512], F32, tag="ost")
                            nc.vector.tensor_copy(osb[:], ops[:])
                            nc.sync.dma_start(
                                dram.ap()[
                                    st * 128 : (st + 1) * 128,
                                    fo * 512 : (fo + 1) * 512,
                                ],
                                osb[:],
                            )

                def oproj_steps(st):
                    # one deferred emission step per PE matmul so the caller
                    # can weave them between exp-throttled score matmuls
                    steps = []
                    for fo in range(2):
                        opr = o_ps.tile([128, 512], F32, tag="o")
                        opi = o_ps.tile([128, 512], F32, tag="o")

                        def mk_mm(ps, w_sb, h2, fo=fo):
                            def go():
                                nc.tensor.matmul(
                                    ps[:], at[:, h2, st * 128 : (st + 1) * 128],
                                    w_sb[:, h2, fo * 512 : (fo + 1) * 512],
                                    start=(h2 == 0), stop=(h2 == HL - 1),
                                )
                            return go

                        for h2 in range(HL):
                            steps.append(mk_mm(opr, wor_sb, h2))
                            steps.append(mk_mm(opi, woi_sb, h2))

                        def mk_epi(opr=opr, opi=opi, fo=fo):
                            def go():
                                for ops, dram in ((opr, out_r), (opi, out_i)):
                                    osb = opool.tile([128, 512], F32, tag="ost")
                                    nc.vector.tensor_copy(osb[:], ops[:])
                                    nc.gpsimd.dma_start(
                                        dram.ap()[
                                            st * 128 : (st + 1) * 128,
                                            fo * 512 : (fo + 1) * 512,
                                        ],
                                        osb[:],
                                    )
                            return go

                        steps.append(mk_epi())
                    return steps

                NBLK = QW // 128
                for qbb in range(S // QW):
                    q0 = qbb * QW
                    for h in range(HL):
                        steps = (
                            oproj_steps((qbb - 1) * NBLK + h) if qbb > 0 else []
                        )
                        p_tiles = []
                        for ktile in range(NST):
                            sps = sc_ps.tile([128, QW], F32, tag="sc")
                            for half in range(QW // 512):
                                nc.tensor.matmul(
                                    sps[:, half * 512 : (half + 1) * 512],
                                    kt[:, h, ktile * 128 : (ktile + 1) * 128],
                                    qt[:, h, q0 + half * 512 : q0 + (half + 1) * 512],
                                )
                            pt = p_pool.tile([128, QW], BF, tag="p")
                            nc.scalar.activation(pt[:], sps[:], EXP, scale=scale)
                            p_tiles.append(pt)
                            # weave ~1.5 oproj steps per exp-throttled score
                            if ktile >= 3:
                                for _ in range(2 if ktile % 2 else 1):
                                    if steps:
                                        steps.pop(0)()
                        for s_fn in steps:
                            s_fn()
                        for qs in range(QW // 128):
                            aps = a_ps.tile([128, D2 + 1], F32, tag="a")
                            for ktile in range(NST):
                                nc.tensor.matmul(
                                    aps[:],
                                    p_tiles[ktile][:, qs * 128 : (qs + 1) * 128],
                                    va[:, h, ktile, :],
                                    start=(ktile == 0), stop=(ktile == NST - 1),
                                )
                            rcp = r_pool.tile([128, 1], F32, tag="r")
                            nc.vector.reciprocal(rcp[:], aps[:, D2 : D2 + 1])
                            asb = a_pool.tile([128, D2], BF, tag="asb")
                            nc.vector.tensor_scalar_mul(asb[:], aps[:, 0:D2], rcp[:])
                            nc.sync.dma_start(
                                at[:, h, q0 + qs * 128 : q0 + (qs + 1) * 128],
                                asb[:],
                                transpose=True,
                            )
                            # last q-block: its own oproj blocks become ready
                            # one-by-one as the final head's at slices land
                            if h == HL - 1 and qbb == S // QW - 1:
                                for s_fn in oproj_steps(qbb * NBLK + qs):
                                    s_fn()

    nc.compile()
    return nc


def _get_program(with_bias=False):
    key = f"nc_bias{with_bias}"
    if key not in _CACHE:
        _CACHE[key] = _build_program(with_bias=with_bias)
    return _CACHE[key]


def _prep_core_inputs(inputs, core):
    """Host-side shard prep for one core (batch b, heads h0..h0+3)."""
    f32 = np.float32
    b = core // (NCORES // B)
    h0 = (core % (NCORES // B)) * HL
    hs = slice(h0 * D, (h0 + HL) * D)  # feature slice of this core's heads

    xr = np.asarray(inputs["x_r"][b], dtype=f32)
    xi = np.asarray(inputs["x_i"][b], dtype=f32)
    xcat = np.concatenate([xr.T, xi.T], axis=0)  # [2F, S]
    xcat = np.ascontiguousarray(xcat)

    def wstack(wr, wi):
        # Karatsuba blocks [F, 3, HL*D]: m0 = Wr, m1 = Wi, m2 = Wr+Wi
        wr = np.asarray(wr, dtype=f32)[:, hs]
        wi = np.asarray(wi, dtype=f32)[:, hs]
        w = np.stack([wr, wi, wr + wi], axis=1)  # [F, 3, HL*D]
        return np.ascontiguousarray(w.reshape(F, 3 * HL * D))

    def brep(br, bi):
        br = np.asarray(br, dtype=f32)[hs].reshape(HL, D)
        bi = np.asarray(bi, dtype=f32)[hs].reshape(HL, D)
        bcat = np.concatenate([br, bi], axis=1).reshape(HL * D2)
        return np.ascontiguousarray(
            np.broadcast_to(bcat, (128, HL * D2)).astype(BF16)
        )

    def wostack(wor, woi):
        # rows r<64 -> wo_top[d], r>=64 -> wo_bot[d]  per head
        wor = np.asarray(wor, dtype=f32)[hs].reshape(HL, D, F)
        woi = np.asarray(woi, dtype=f32)[hs].reshape(HL, D, F)
        w = np.empty((HL, D2, F), dtype=f32)
        w[:, :D] = wor
        w[:, D:] = woi
        return np.ascontiguousarray(w.reshape(HL * D2, F).astype(BF16))

    mask = np.asarray(inputs["mask"][b], dtype=f32)
    mask_cols = np.ascontiguousarray(mask.reshape(NST, 128).T)

    return {
        "xcat": xcat,
        "wq": wstack(inputs["Wq_r"], inputs["Wq_i"]),
        "wk": wstack(inputs["Wk_r"], inputs["Wk_i"]),
        "wv": wstack(inputs["Wv_r"], inputs["Wv_i"]),
        "wor": wostack(inputs["Wo_r"], -np.asarray(inputs["Wo_i"], dtype=f32)),
        "woi": wostack(inputs["Wo_i"], inputs["Wo_r"]),
        "bqrep": brep(inputs["bq_r"], inputs["bq_i"]),
        "bkrep": brep(inputs["bk_r"], inputs["bk_i"]),
        "bvrep": brep(inputs["bv_r"], inputs["bv_i"]),
        "maskcols": mask_cols,
        "ident": np.eye(128, dtype=f32),
    }


def kernel(_trace=False, _trace_kwargs=None, **inputs):
    from concourse.bass_utils import run_bass_kernel_spmd

    with_bias = any(
        np.any(np.asarray(inputs[k]))
        for k in ("bq_r", "bq_i", "bk_r", "bk_i", "bv_r", "bv_i")
    )
    nc = _get_program(with_bias=bool(with_bias))
    in_maps = [_prep_core_inputs(inputs, c) for c in range(NCORES)]
    res = run_bass_kernel_spmd(
        nc, in_maps, core_ids=list(range(NCORES)),
        trace=_trace, **(_trace_kwargs or {}),
    )
    _CACHE["last_results"] = res

    bo_r = np.asarray(inputs["bo_r"], dtype=np.float32)
    bo_i = np.asarray(inputs["bo_i"], dtype=np.float32)
    out = np.empty((2, B, S, F), dtype=np.float32)
    cpb = NCORES // B
    for b in range(B):
        cores = range(b * cpb, (b + 1) * cpb)
        out[0, b] = sum(res.results[c]["out_r"] for c in cores) + bo_r
        out[1, b] = sum(res.results[c]["out_i"] for c in cores) + bo_i
    return out

